# revision 15
# baseline (speedup 1.0000x reference)
"""Trainium2 Bass kernel for DiffusionOperator (polynomial graph diffusion).

result = sum_k coeffs[k] * T^k x,  T = D^-1/2 A D^-1/2 (deg by edge col),
coeffs = softmax(MLP(graph stats)).

Strategy (8 NeuronCores, SPMD), v2:
  * Nodes partitioned into 8 contiguous slices of R=12500 (dest side).
  * Reformulation: s_0 = dis*x; a_k = A @ s_{k-1}; s_k = dis^2 * a_k;
    result = c0*x + (sum_k c_k s_k)/dis. Per-edge work is a pure gather +
    one-hot matmul segment-sum (bf16 on the PE; fp32 PSUM accumulation).
  * The replicated source table is split into 4 row-chunks (each chunk =
    the union over cores of a block-aligned quarter of their rows). Each
    chunk has its own AllGather, so chunk AGs launch as soon as their dest
    blocks finish — overlapping the collective with compute, and next
    step's gathers start per-chunk.
  * Gather indices are identical across steps; within each (core, chunk,
    dest-block) group edges are sorted by source address for DMA locality.
  * dma_gather pieces of 2048 idxs on 4 SWDGE queues; descriptor carveout
    enlarged to 48KB so Pool desc-gen runs ahead of the DMA engines.

Self-contained: hardcodes full-problem shapes; builds/compiles on first call.
"""

import math
import os
import sys
from dataclasses import dataclass, field

import numpy as np

for _p in ("/opt/trn_rl_repo",):
    if _p not in sys.path:
        sys.path.insert(0, _p)

import concourse.bacc as bacc
import concourse.bass as bass
import concourse.bass_isa as bass_isa
import concourse.mybir as mybir
import concourse.tile as tile
from concourse import tile_sem_assignment as _tsa
from concourse.masks import make_identity
from concourse.tile_scheduler import DMAInst as _DMAInst


def _install_queue_aware_dmasw():
    """Map Pool SWDGE DMAs to DMASW lanes by queue_num (lane = q + 4*(i%2))
    so multi-queue dma_gather passes the per-queue semaphore-lock check."""
    if getattr(_tsa.TileClockTick, "_qaware", False):
        return
    orig = _tsa.TileClockTick._assign_tick

    def patched(self, inst):
        if (
            isinstance(inst, _DMAInst)
            and inst.engine == mybir.EngineType.Pool
            and not isinstance(inst, bass_isa.UserSyncedRemoteDMADescs)
        ):
            qn = int(getattr(inst, "queue_num", 0) or 0)
            ctr = self.__dict__.setdefault("_qctr", {})
            c = ctr.get(qn, 0)
            ctr[qn] = c + 1
            self.next_sw_dma_idx = qn + 4 * (c % 2)
        return orig(self, inst)

    _tsa.TileClockTick._assign_tick = patched
    _tsa.TileClockTick._qaware = True


_install_queue_aware_dmasw()

F32 = mybir.dt.float32
BF16 = mybir.dt.bfloat16
I16 = mybir.dt.int16
AF = mybir.ActivationFunctionType
ALU = mybir.AluOpType
P = 128
NSC = 4  # source chunks (== n sub-AllGathers == n SWDGE queues)


@dataclass(frozen=True)
class Cfg:
    N: int          # nodes
    E: int          # edges
    C: int          # channels (64)
    H: int          # mlp hidden (32)
    K: int          # poly degree (5)
    ncores: int     # 8
    piece_tok: int  # idxs per dma_gather piece
    scratch: int    # SWDGE descriptor carveout bytes
    c_need: int     # 128-token chunks per (core, src-chunk, dest-block)

    @property
    def R(self):  # rows per core
        return self.N // self.ncores

    @property
    def NB(self):  # dest blocks per core
        return math.ceil(self.R / P)

    @property
    def tail(self):  # real rows in last block
        return self.R - (self.NB - 1) * P

    @property
    def chunk_blocks(self):  # NB split into NSC block-aligned chunks
        base, rem = divmod(self.NB, NSC)
        return [base + 1] * rem + [base] * (NSC - rem)

    @property
    def chunk_rows(self):  # real rows per chunk (tail only in last)
        cb = self.chunk_blocks
        rows = [b * P for b in cb]
        rows[-1] = self.R - sum(cb[:-1]) * P
        return rows

    @property
    def chunk_start_blocks(self):
        cb = self.chunk_blocks
        return [sum(cb[:i]) for i in range(NSC)]

    @property
    def chunk_start_rows(self):
        return [sb * P for sb in self.chunk_start_blocks]

    @property
    def cap(self):  # tokens per (src-chunk, dest-block)
        return self.c_need * P

    @property
    def ncc(self):  # chunk columns per dest block
        return NSC * self.c_need

    @property
    def LQ(self):  # tokens per (core, src-chunk) stream
        return self.NB * self.cap

    @property
    def LQP(self):  # padded stream length (x piece_tok)
        return math.ceil(self.LQ / self.piece_tok) * self.piece_tok

    @property
    def n_pieces(self):
        return self.LQP // self.piece_tok

    @property
    def nchunkcol(self):  # total chunk columns in dcols
        return self.NB * self.ncc


FULL = dict(
    N=100000, E=1600000, C=64, H=32, K=5, ncores=8, piece_tok=1024, scratch=32768
)


def _preprocess(x, edge_index, cfg_kw):
    """Host-side index preprocessing -> per-core input maps + Cfg."""
    N, E, ncores = cfg_kw["N"], cfg_kw["E"], cfg_kw["ncores"]
    R = N // ncores
    NB = math.ceil(R / P)
    base, rem = divmod(NB, NSC)
    cblocks = [base + 1] * rem + [base] * (NSC - rem)
    cstartb = [sum(cblocks[:i]) for i in range(NSC)]
    cstartr = np.array([sb * P for sb in cstartb], dtype=np.int64)
    crows = [b * P for b in cblocks]
    crows[-1] = R - sum(cblocks[:-1]) * P
    crows = np.array(crows, dtype=np.int64)

    row = np.asarray(edge_index[0], dtype=np.int64)
    col = np.asarray(edge_index[1], dtype=np.int64)
    deg = np.bincount(col, minlength=N).astype(np.float32)

    m = row // R
    b = (row % R) // P
    dl = (row % R) % P
    mo = col // R
    lr = col % R
    csrc = np.digitize(lr, cstartr[1:])  # 0..NSC-1
    lidx = (mo * crows[csrc] + (lr - cstartr[csrc])).astype(np.int16)

    ngroups = ncores * NSC * NB
    key = (m * NSC + csrc) * NB + b
    order = np.lexsort((lidx, key))  # group-major, src-address-sorted
    counts = np.bincount(key, minlength=ngroups)
    c_need = int(math.ceil(counts.max() / P))
    cap = c_need * P

    starts = np.zeros(ngroups, dtype=np.int64)
    np.cumsum(counts[:-1], out=starts[1:])
    rank = np.arange(E, dtype=np.int64) - np.repeat(starts, counts)
    pos = np.repeat(np.arange(ngroups, dtype=np.int64) * cap, counts) + rank

    idx_all = np.zeros(ngroups * cap, dtype=np.int16)
    dst_all = np.full(ngroups * cap, 255.0, dtype=np.float32)
    idx_all[pos] = lidx[order]
    dst_all[pos] = dl[order].astype(np.float32)

    cfg = Cfg(c_need=c_need, **cfg_kw)
    LQ, LQP = cfg.LQ, cfg.LQP

    # dcols: [(m), P, NB*ncc] with column index gb*ncc + c*c_need + j
    dst_all = (
        dst_all.reshape(ncores, NSC, NB, c_need, P)
        .transpose(0, 2, 1, 3, 4)
        .reshape(ncores, NB * NSC * c_need, P)
    )
    idx_all = idx_all.reshape(ncores, NSC, LQ)
    if LQP > LQ:
        idx_all = np.concatenate(
            [idx_all, np.zeros((ncores, NSC, LQP - LQ), np.int16)], axis=2
        )

    in_maps = []
    for mm in range(ncores):
        # wrapped int16 indices: token i of stream c -> [i%16, i//16]; replicate
        # the 16-partition pattern across all 128 partitions (8 gpsimd cores).
        wr = np.concatenate(
            [
                np.tile(idx_all[mm, cc].reshape(LQP // 16, 16).T, (8, 1))
                for cc in range(NSC)
            ],
            axis=1,
        )  # [128, NSC*LQP//16]
        degp = np.ones(NB * P, dtype=np.float32)
        degp[:R] = deg[mm * R : (mm + 1) * R]
        in_maps.append(
            {
                "xs": np.ascontiguousarray(x[mm * R : (mm + 1) * R]).astype(np.float32),
                "degp": np.ascontiguousarray(degp.reshape(NB, P).T),
                "dcols": np.ascontiguousarray(dst_all[mm].T).astype(
                    np.float32
                ),  # converted to bf16 tile on device load? no — host bf16 below
                "idx": np.ascontiguousarray(wr),
            }
        )
    # dcols must be bf16 on device; ml_dtypes for host conversion
    try:
        from ml_dtypes import bfloat16 as _bf16

        for im in in_maps:
            im["dcols"] = im["dcols"].astype(_bf16)
    except ImportError:
        # fall back: keep f32 tensor param (device tile declared f32 then?)
        raise
    return cfg, in_maps


def _build_program(cfg: Cfg):
    nc = bacc.Bacc(
        "TRN2",
        num_swdge_queues=NSC,
        dynamic_dma_scratch_size=cfg.scratch,
    )
    C, NB, R, K = cfg.C, cfg.NB, cfg.R, cfg.K
    cneed, cap, ncc = cfg.c_need, cfg.cap, cfg.ncc
    cblocks = cfg.chunk_blocks
    cstartb = cfg.chunk_start_blocks
    crows = cfg.chunk_rows
    cstartr = cfg.chunk_start_rows

    xs_t = nc.declare_dram_parameter("xs", [R, C], F32, isOutput=False)
    degp_t = nc.declare_dram_parameter("degp", [P, NB], F32, isOutput=False)
    dcols_t = nc.declare_dram_parameter("dcols", [P, cfg.nchunkcol], BF16, isOutput=False)
    idx_t = nc.declare_dram_parameter(
        "idx", [P, NSC * (cfg.LQP // 16)], I16, isOutput=False
    )
    w1t_t = nc.declare_dram_parameter("w1t", [C + 4, cfg.H], F32, isOutput=False)
    b1c_t = nc.declare_dram_parameter("b1c", [cfg.H, 1], F32, isOutput=False)
    w2t_t = nc.declare_dram_parameter("w2t", [cfg.H, K + 1], F32, isOutput=False)
    b2r_t = nc.declare_dram_parameter("b2r", [1, K + 1], F32, isOutput=False)
    out_t = nc.declare_dram_parameter("out", [R, C], F32, isOutput=True)

    # per-step, per-chunk local slices and replicated tables (s_K lives in
    # SBUF only — the final combine is fused into step K)
    s_chk = [
        [nc.dram_tensor(f"s{k}c{c}", [crows[c], C], F32) for c in range(NSC)]
        for k in range(K)
    ]
    table = [
        [
            nc.dram_tensor(
                f"tbl{k}c{c}", [cfg.ncores * crows[c], C], F32, addr_space="Shared"
            )
            for c in range(NSC)
        ]
        for k in range(K)
    ]
    stats_loc = nc.dram_tensor("stats_loc", [136], F32)
    stats_red = nc.dram_tensor("stats_red", [136], F32, addr_space="Shared")

    groups = [list(range(cfg.ncores))]

    def chunk_views(t, c):
        """[crows[c], C] dram slice of t viewed p-major (+ tail view if any)."""
        nfull = crows[c] // P
        lo = cstartr[c]
        main = None
        if nfull > 0:
            main = t[lo : lo + nfull * P, :].rearrange("(b p) f -> p b f", p=P)
        tl = None
        if crows[c] > nfull * P:
            tl = t[lo + nfull * P : lo + crows[c], :]
        return main, tl, nfull

    def own_views(t, c):
        """views of a per-chunk dram tensor [crows[c], C]."""
        nfull = crows[c] // P
        main = None
        if nfull > 0:
            main = t[0 : nfull * P, :].rearrange("(b p) f -> p b f", p=P)
        tl = None
        if crows[c] > nfull * P:
            tl = t[nfull * P : crows[c], :]
        return main, tl, nfull

    with tile.TileContext(nc) as tc:
        with (
            tc.tile_pool(name="const", bufs=1) as cpool,
            tc.tile_pool(name="stage", bufs=8) as stpool,
            tc.tile_pool(name="graw", bufs=6) as gpool,
            tc.tile_pool(name="gbf", bufs=12) as gbpool,
            tc.tile_pool(name="sp", bufs=3) as spool,
            tc.tile_pool(name="small", bufs=2) as smpool,
            tc.tile_pool(name="pmain", bufs=4, space="PSUM") as pmain,
            tc.tile_pool(name="psmall", bufs=2, space="PSUM") as psmall,
        ):
            # ---- constants ----
            iota_t = cpool.tile([P, ncc * P], BF16)
            nc.gpsimd.iota(
                iota_t[:],
                [[0, ncc], [1, P]],
                channel_multiplier=0,
                allow_small_or_imprecise_dtypes=True,
            )
            ident64 = cpool.tile([C, C], F32)
            make_identity(nc, ident64[:])
            ones_col = cpool.tile([P, 1], F32)
            nc.gpsimd.memset(ones_col[:], 1.0)
            ones_row = cpool.tile([1, P], F32)
            nc.gpsimd.memset(ones_row[:], 1.0)

            idxs = cpool.tile([P, NSC * (cfg.LQP // 16)], I16)
            nc.sync.dma_start(out=idxs[:], in_=idx_t[:])
            dcols = cpool.tile([P, cfg.nchunkcol], BF16)
            nc.sync.dma_start(out=dcols[:], in_=dcols_t[:])
            degp = cpool.tile([P, NB], F32)
            nc.sync.dma_start(out=degp[:], in_=degp_t[:])
            w1t = cpool.tile([C + 4, cfg.H], F32)
            nc.sync.dma_start(out=w1t[:], in_=w1t_t[:])
            b1c = cpool.tile([cfg.H, 1], F32)
            nc.sync.dma_start(out=b1c[:], in_=b1c_t[:])
            w2t = cpool.tile([cfg.H, K + 1], F32)
            nc.sync.dma_start(out=w2t[:], in_=w2t_t[:])
            b2r = cpool.tile([1, K + 1], F32)
            nc.sync.dma_start(out=b2r[:], in_=b2r_t[:])

            # dis = min(deg^-0.5, 1e6); dis2 = dis^2; rdis = 1/dis
            dis = cpool.tile([P, NB], F32)
            nc.scalar.activation(dis[:], degp[:], AF.Sqrt)
            nc.vector.tensor_scalar_max(dis[:], dis[:], 1.0e-6)
            nc.vector.reciprocal(dis[:], dis[:])
            dis2 = cpool.tile([P, NB], F32)
            nc.vector.tensor_tensor(dis2[:], dis[:], dis[:], op=ALU.mult)
            rdis = cpool.tile([P, NB], F32)
            nc.vector.reciprocal(rdis[:], dis[:])

            # ---- phase A: per chunk load x, stats partials, s0, AG ----
            csum_ps = psmall.tile([P, C], F32, tag="sm")
            sq_ps = psmall.tile([P, C], F32, tag="sm2")
            for c in range(NSC):
                nb_c = cblocks[c]
                x_c = stpool.tile([P, nb_c * C], F32, tag="stage")
                xm, xtl, nfull = chunk_views(xs_t, c)
                if nfull < nb_c:
                    nc.gpsimd.memset(x_c[:, nfull * C :], 0.0)
                if nfull > 0:
                    nc.sync.dma_start(
                        out=x_c[:, 0 : nfull * C].rearrange("p (b f) -> p b f", f=C),
                        in_=xm,
                    )
                if xtl is not None:
                    nc.sync.dma_start(
                        out=x_c[0 : cfg.tail, nfull * C : (nfull + 1) * C], in_=xtl
                    )
                s_c = stpool.tile([P, nb_c * C], F32, tag="stage")
                for lb in range(nb_c):
                    gb = cstartb[c] + lb
                    nc.tensor.matmul(
                        csum_ps[0:C, 0:1],
                        lhsT=x_c[:, lb * C : (lb + 1) * C],
                        rhs=ones_col[:],
                        start=(gb == 0),
                        stop=(gb == NB - 1),
                    )
                    nc.tensor.matmul(
                        sq_ps[0:C, 0:C],
                        lhsT=x_c[:, lb * C : (lb + 1) * C],
                        rhs=x_c[:, lb * C : (lb + 1) * C],
                        start=(gb == 0),
                        stop=(gb == NB - 1),
                    )
                    # s0 = dis * x
                    nc.scalar.activation(
                        s_c[:, lb * C : (lb + 1) * C],
                        x_c[:, lb * C : (lb + 1) * C],
                        AF.Copy,
                        scale=dis[:, gb : gb + 1],
                    )
                sm, stl, nfull2 = own_views(s_chk[0][c], c)
                if nfull2 > 0:
                    nc.sync.dma_start(
                        out=sm,
                        in_=s_c[:, 0 : nfull2 * C].rearrange("p (b f) -> p b f", f=C),
                    )
                if stl is not None:
                    nc.sync.dma_start(
                        out=stl, in_=s_c[0 : cfg.tail, nfull2 * C : (nfull2 + 1) * C]
                    )
                nc.gpsimd.collective_compute(
                    "AllGather",
                    ALU.bypass,
                    replica_groups=groups,
                    ins=[s_chk[0][c][:]],
                    outs=[table[0][c][:]],
                )

            csum_sb = smpool.tile([C, 1], F32)
            nc.vector.tensor_copy(csum_sb[:], csum_ps[0:C, 0:1])
            sqd = smpool.tile([C, C], F32)
            nc.vector.tensor_tensor(sqd[:], sq_ps[0:C, 0:C], ident64[:], op=ALU.mult)
            sqch = smpool.tile([C, 1], F32)
            nc.vector.tensor_reduce(sqch[:], sqd[:], axis=mybir.AxisListType.X, op=ALU.add)

            zpad = smpool.tile([1, 8], F32, tag="zp")
            nc.gpsimd.memset(zpad[:], 0.0)
            nc.sync.dma_start(out=stats_loc[0:C], in_=csum_sb[:])
            nc.sync.dma_start(out=stats_loc[C : 2 * C], in_=sqch[:])
            nc.sync.dma_start(out=stats_loc[2 * C : 2 * C + 8], in_=zpad[:])
            nc.gpsimd.collective_compute(
                "AllReduce",
                ALU.add,
                replica_groups=groups,
                ins=[stats_loc[:]],
                outs=[stats_red[:]],
            )

            # ---- coeff MLP (overlaps diffusion steps) ----
            red = smpool.tile([1, 136], F32)
            nc.sync.dma_start(out=red[:], in_=stats_red[:])
            cin = smpool.tile([P, 1], F32, tag="cin")
            nc.sync.dma_start(out=cin[0:C, 0:1], in_=red[0:1, 0:C])
            nc.vector.tensor_scalar_mul(cin[0:C, 0:1], cin[0:C, 0:1], 1.0 / cfg.N)
            M = float(cfg.N * cfg.C)
            mean = smpool.tile([1, 1], F32, tag="m1")
            nc.vector.tensor_reduce(
                mean[:], red[0:1, 0:C], axis=mybir.AxisListType.X, op=ALU.add
            )
            nc.scalar.mul(mean[:], mean[:], 1.0 / M)
            sqred = smpool.tile([1, 1], F32, tag="m2")
            nc.vector.tensor_reduce(
                sqred[:], red[0:1, C : 2 * C], axis=mybir.AxisListType.X, op=ALU.add
            )
            msq = smpool.tile([1, 1], F32, tag="m3")
            nc.vector.tensor_tensor(msq[:], mean[:], mean[:], op=ALU.mult)
            nc.scalar.mul(msq[:], msq[:], -M)
            nc.vector.tensor_tensor(msq[:], sqred[:], msq[:], op=ALU.add)
            nc.scalar.mul(msq[:], msq[:], 1.0 / (M - 1.0))
            nc.scalar.activation(msq[:], msq[:], AF.Sqrt)  # std
            srow = smpool.tile([1, 4], F32, tag="m4")
            nc.vector.tensor_copy(srow[0:1, 0:1], mean[:])
            nc.vector.tensor_copy(srow[0:1, 1:2], msq[:])
            nc.gpsimd.memset(srow[0:1, 2:3], float(cfg.N))
            nc.gpsimd.memset(srow[0:1, 3:4], float(cfg.E))
            nc.gpsimd.dma_start(out=cin[C : C + 4, 0:1], in_=srow[:])

            h_ps = psmall.tile([P, C], F32, tag="sm")
            nc.tensor.matmul(
                h_ps[0 : cfg.H, 0:1], lhsT=w1t[:], rhs=cin[0 : C + 4, 0:1],
                start=True, stop=True,
            )
            h_sb = smpool.tile([cfg.H, 1], F32, tag="h")
            nc.scalar.activation(h_sb[:], h_ps[0 : cfg.H, 0:1], AF.Relu, bias=b1c[:])
            c_ps = psmall.tile([P, C], F32, tag="sm")
            nc.tensor.matmul(
                c_ps[0:1, 0 : K + 1], lhsT=h_sb[:], rhs=w2t[:], start=True, stop=True
            )
            z = smpool.tile([1, K + 1], F32, tag="z")
            nc.vector.tensor_tensor(z[:], c_ps[0:1, 0 : K + 1], b2r[:], op=ALU.add)
            zmax = smpool.tile([1, 1], F32, tag="m5")
            nc.vector.tensor_reduce(zmax[:], z[:], axis=mybir.AxisListType.X, op=ALU.max)
            nc.vector.tensor_scalar(
                z[:], z[:], zmax[0:1, 0:1], None, op0=ALU.subtract
            )
            nc.scalar.activation(z[:], z[:], AF.Exp)
            zsum = smpool.tile([1, 1], F32, tag="m6")
            nc.vector.tensor_reduce(zsum[:], z[:], axis=mybir.AxisListType.X, op=ALU.add)
            nc.vector.reciprocal(zsum[:], zsum[:])
            nc.vector.tensor_scalar_mul(z[:], z[:], zsum[0:1, 0:1])
            cb_ps = psmall.tile([P, C], F32, tag="sm")
            nc.tensor.matmul(
                cb_ps[:, 0 : K + 1], lhsT=ones_row[:], rhs=z[:], start=True, stop=True
            )
            c_bc = cpool.tile([P, K + 1], F32)
            nc.vector.tensor_copy(c_bc[:], cb_ps[:, 0 : K + 1])

            # ---- phase B: K diffusion steps ----
            LQ16 = cfg.LQP // 16
            pt16 = cfg.piece_tok // 16
            pt_sub = cfg.piece_tok // P  # 128-token sub-chunks per piece
            for k in range(1, K + 1):
                # gathers: i-major across chunks to match consumption order
                pieces = [[None] * cfg.n_pieces for _ in range(NSC)]
                for i in range(cfg.n_pieces):
                    for c in range(NSC):
                        gt = gpool.tile([P, pt_sub * C], F32, tag="g")
                        nc.gpsimd.dma_gather(
                            gt[:].rearrange("p (s f) -> p s f", f=C),
                            table[k - 1][c][:],
                            idxs[:, c * LQ16 + i * pt16 : c * LQ16 + (i + 1) * pt16],
                            num_idxs=cfg.piece_tok,
                            num_idxs_reg=cfg.piece_tok,
                            elem_size=C,
                            queue_num=c,
                        )
                        gb16 = gbpool.tile([P, pt_sub * C], BF16, tag="gb")
                        nc.scalar.activation(gb16[:], gt[:], AF.Copy)
                        pieces[c][i] = gb16

                for c in range(NSC):
                    nb_c = cblocks[c]
                    s_c = stpool.tile([P, nb_c * C], F32, tag="stage")
                    for lb in range(nb_c):
                        gb = cstartb[c] + lb
                        ps = pmain.tile([P, C], F32, tag="ps")
                        S = spool.tile([P, ncc * P], BF16, tag="S")
                        nc.vector.tensor_tensor(
                            S[:].rearrange("p (q f) -> p q f", f=P),
                            dcols[:, gb * ncc : (gb + 1) * ncc].to_broadcast(
                                [P, ncc, P]
                            ),
                            iota_t[:].rearrange("p (q f) -> p q f", f=P),
                            op=ALU.is_equal,
                        )
                        for sc in range(NSC):
                            for j in range(cneed):
                                t0 = gb * cap + j * P
                                gt = pieces[sc][t0 // cfg.piece_tok]
                                gv = gt[:].rearrange("p (s f) -> p s f", f=C)
                                nc.tensor.matmul(
                                    ps[:],
                                    lhsT=S[
                                        :,
                                        (sc * cneed + j) * P : (sc * cneed + j + 1) * P,
                                    ],
                                    rhs=gv[:, (t0 % cfg.piece_tok) // P, :],
                                    start=(sc == 0 and j == 0),
                                    stop=(sc == NSC - 1 and j == cneed - 1),
                                )
                        # s_k = dis^2 * a_k
                        nc.scalar.activation(
                            s_c[:, lb * C : (lb + 1) * C],
                            ps[:],
                            AF.Copy,
                            scale=dis2[:, gb : gb + 1],
                        )
                    if k < K:
                        sm, stl, nfull = own_views(s_chk[k][c], c)
                        if nfull > 0:
                            nc.sync.dma_start(
                                out=sm,
                                in_=s_c[:, 0 : nfull * C].rearrange(
                                    "p (b f) -> p b f", f=C
                                ),
                            )
                        if stl is not None:
                            nc.sync.dma_start(
                                out=stl,
                                in_=s_c[0 : cfg.tail, nfull * C : (nfull + 1) * C],
                            )
                        nc.gpsimd.collective_compute(
                            "AllGather",
                            ALU.bypass,
                            replica_groups=groups,
                            ins=[s_chk[k][c][:]],
                            outs=[table[k][c][:]],
                        )
                        continue

                    # ---- k == K: fuse the final combine for this chunk ----
                    acc = stpool.tile([P, nb_c * C], F32, tag="stage")
                    nc.vector.tensor_scalar_mul(acc[:], s_c[:], c_bc[:, K : K + 1])
                    for kk in range(1, K):
                        sl = stpool.tile([P, nb_c * C], F32, tag="stage")
                        sm, stl, nfull = own_views(s_chk[kk][c], c)
                        if nfull < nb_c:
                            nc.gpsimd.memset(sl[:, nfull * C :], 0.0)
                        if nfull > 0:
                            nc.sync.dma_start(
                                out=sl[:, 0 : nfull * C].rearrange(
                                    "p (b f) -> p b f", f=C
                                ),
                                in_=sm,
                            )
                        if stl is not None:
                            nc.sync.dma_start(
                                out=sl[0 : cfg.tail, nfull * C : (nfull + 1) * C],
                                in_=stl,
                            )
                        nc.vector.tensor_scalar_mul(sl[:], sl[:], c_bc[:, kk : kk + 1])
                        nc.vector.tensor_tensor(acc[:], acc[:], sl[:], op=ALU.add)
                    for lb in range(nb_c):
                        gb = cstartb[c] + lb
                        nc.vector.tensor_scalar_mul(
                            acc[:, lb * C : (lb + 1) * C],
                            acc[:, lb * C : (lb + 1) * C],
                            rdis[:, gb : gb + 1],
                        )
                    xl = stpool.tile([P, nb_c * C], F32, tag="stage")
                    xm, xtl, nfull = chunk_views(xs_t, c)
                    if nfull < nb_c:
                        nc.gpsimd.memset(xl[:, nfull * C :], 0.0)
                    if nfull > 0:
                        nc.sync.dma_start(
                            out=xl[:, 0 : nfull * C].rearrange("p (b f) -> p b f", f=C),
                            in_=xm,
                        )
                    if xtl is not None:
                        nc.sync.dma_start(
                            out=xl[0 : cfg.tail, nfull * C : (nfull + 1) * C], in_=xtl
                        )
                    nc.vector.tensor_scalar_mul(xl[:], xl[:], c_bc[:, 0:1])
                    nc.vector.tensor_tensor(acc[:], acc[:], xl[:], op=ALU.add)

                    om, otl, nfull = chunk_views(out_t, c)
                    if nfull > 0:
                        nc.sync.dma_start(
                            out=om,
                            in_=acc[:, 0 : nfull * C].rearrange("p (b f) -> p b f", f=C),
                        )
                    if otl is not None:
                        nc.sync.dma_start(
                            out=otl, in_=acc[0 : cfg.tail, nfull * C : (nfull + 1) * C]
                        )

    nc.finalize()
    return nc


_CACHE = {}


def _get_program(cfg: Cfg):
    if cfg not in _CACHE:
        _CACHE[cfg] = _build_program(cfg)
    return _CACHE[cfg]


def _run(inputs, trace=False, cfg_kw=None):
    from concourse.bass_utils import run_bass_kernel_spmd

    cfg_kw = dict(cfg_kw or FULL)
    x = np.asarray(inputs["x"], dtype=np.float32)
    cfg, in_maps = _preprocess(x, inputs["edge_index"], cfg_kw)
    W1 = np.asarray(inputs["W1"], dtype=np.float32)
    b1 = np.asarray(inputs["b1"], dtype=np.float32)
    W2 = np.asarray(inputs["W2"], dtype=np.float32)
    b2 = np.asarray(inputs["b2"], dtype=np.float32)
    for im in in_maps:
        im["w1t"] = np.ascontiguousarray(W1.T)
        im["b1c"] = np.ascontiguousarray(b1[:, None])
        im["w2t"] = np.ascontiguousarray(W2.T)
        im["b2r"] = np.ascontiguousarray(b2[None, :])
    nc = _get_program(cfg)
    res = run_bass_kernel_spmd(
        nc, in_maps, core_ids=list(range(cfg.ncores)), trace=trace
    )
    out = np.concatenate([res.results[i]["out"] for i in range(cfg.ncores)], axis=0)
    return out, res.exec_time_ns


def kernel(**inputs) -> np.ndarray:
    out, _ = _run(inputs)
    return out


# ---------------------------------------------------------------------------
# toy-scale validation against a numpy port of the reference, via CoreSim
# ---------------------------------------------------------------------------


def _np_reference(x, edge_index, W1, b1, W2, b2, K=5):
    N, C = x.shape
    E = edge_index.shape[1]
    row, col = edge_index[0].astype(np.int64), edge_index[1].astype(np.int64)
    deg = np.bincount(col, minlength=N).astype(np.float32)
    with np.errstate(divide="ignore"):
        dis = np.minimum(deg ** -0.5, 1e6).astype(np.float32)
    norm = dis[row] * dis[col]
    xm = x.mean(axis=0)
    stats = np.array([x.mean(), x.std(ddof=1), N, E], dtype=np.float32)
    cin = np.concatenate([xm, stats])
    h = np.maximum(W1 @ cin + b1, 0.0)
    zz = W2 @ h + b2
    zz = np.exp(zz - zz.max())
    coeffs = zz / zz.sum()
    result = coeffs[0] * x
    tx = x.copy()
    for k in range(1, K + 1):
        nt = np.zeros_like(tx)
        np.add.at(nt, row, norm[:, None] * tx[col])
        tx = nt
        result = result + coeffs[k] * tx
    return result


def _selftest_sim():
    from concourse.bass_interp import MultiCoreSim

    rng = np.random.default_rng(0)
    kw = dict(
        N=6400, E=25600, C=64, H=32, K=5, ncores=8, piece_tok=256, scratch=16384
    )
    x = rng.standard_normal((kw["N"], kw["C"])).astype(np.float32)
    ei = rng.integers(0, kw["N"], size=(2, kw["E"])).astype(np.int32)
    W1 = rng.uniform(-1, 1, (kw["H"], kw["C"] + 4)).astype(np.float32) / 8
    b1 = rng.uniform(-1, 1, (kw["H"],)).astype(np.float32) / 8
    W2 = rng.uniform(-1, 1, (kw["K"] + 1, kw["H"])).astype(np.float32) / 5
    b2 = rng.uniform(-1, 1, (kw["K"] + 1,)).astype(np.float32) / 5

    cfg, in_maps = _preprocess(x, ei, kw)
    print("toy cfg:", cfg)
    for im in in_maps:
        im["w1t"] = np.ascontiguousarray(W1.T)
        im["b1c"] = np.ascontiguousarray(b1[:, None])
        im["w2t"] = np.ascontiguousarray(W2.T)
        im["b2r"] = np.ascontiguousarray(b2[None, :])
    nc = _build_program(cfg)
    sim = MultiCoreSim(nc, cfg.ncores)
    for i in range(cfg.ncores):
        for name, arr in in_maps[i].items():
            sim.cores[i].tensor(name)[:] = arr
    sim.simulate()
    out = np.concatenate(
        [sim.cores[i].tensor("out") for i in range(cfg.ncores)], axis=0
    )
    exp = _np_reference(x, ei, W1, b1, W2, b2, K=kw["K"])
    err = np.abs(out - exp).max() / (np.abs(exp).max() + 1e-30)
    rel = np.linalg.norm(out - exp) / (np.linalg.norm(exp) + 1e-30)
    print(f"sim selftest: max-abs-rel {err:.3e}  fro-rel {rel:.3e}")
    assert rel < 5e-3, (rel, err)
    print("SIM SELFTEST PASSED")


if __name__ == "__main__":
    _selftest_sim()


# revision 16
# speedup vs baseline: 1.1053x; 1.1053x over previous
"""Trainium2 Bass kernel for DiffusionOperator (polynomial graph diffusion).

result = sum_k coeffs[k] * T^k x,  T = D^-1/2 A D^-1/2 (deg by edge col),
coeffs = softmax(MLP(graph stats)).

Strategy (8 NeuronCores, SPMD), v2:
  * Nodes partitioned into 8 contiguous slices of R=12500 (dest side).
  * Reformulation: s_0 = dis*x; a_k = A @ s_{k-1}; s_k = dis^2 * a_k;
    result = c0*x + (sum_k c_k s_k)/dis. Per-edge work is a pure gather +
    one-hot matmul segment-sum (bf16 on the PE; fp32 PSUM accumulation).
  * The replicated source table is split into 4 row-chunks (each chunk =
    the union over cores of a block-aligned quarter of their rows). Each
    chunk has its own AllGather, so chunk AGs launch as soon as their dest
    blocks finish — overlapping the collective with compute, and next
    step's gathers start per-chunk.
  * Gather indices are identical across steps; within each (core, chunk,
    dest-block) group edges are sorted by source address for DMA locality.
  * dma_gather pieces of 2048 idxs on 4 SWDGE queues; descriptor carveout
    enlarged to 48KB so Pool desc-gen runs ahead of the DMA engines.

Self-contained: hardcodes full-problem shapes; builds/compiles on first call.
"""

import math
import os
import sys
from dataclasses import dataclass, field

import numpy as np

for _p in ("/opt/trn_rl_repo",):
    if _p not in sys.path:
        sys.path.insert(0, _p)

import concourse.bacc as bacc
import concourse.bass as bass
import concourse.bass_isa as bass_isa
import concourse.mybir as mybir
import concourse.tile as tile
from concourse import tile_sem_assignment as _tsa
from concourse.masks import make_identity
from concourse.tile_scheduler import DMAInst as _DMAInst


def _install_queue_aware_dmasw():
    """Map Pool SWDGE DMAs to DMASW lanes by queue_num (lane = q + 4*(i%2))
    so multi-queue dma_gather passes the per-queue semaphore-lock check."""
    if getattr(_tsa.TileClockTick, "_qaware", False):
        return
    orig = _tsa.TileClockTick._assign_tick

    def patched(self, inst):
        if (
            isinstance(inst, _DMAInst)
            and inst.engine == mybir.EngineType.Pool
            and not isinstance(inst, bass_isa.UserSyncedRemoteDMADescs)
        ):
            qn = int(getattr(inst, "queue_num", 0) or 0)
            ctr = self.__dict__.setdefault("_qctr", {})
            c = ctr.get(qn, 0)
            ctr[qn] = c + 1
            self.next_sw_dma_idx = qn + 4 * (c % 2)
        return orig(self, inst)

    _tsa.TileClockTick._assign_tick = patched
    _tsa.TileClockTick._qaware = True


_install_queue_aware_dmasw()

F32 = mybir.dt.float32
BF16 = mybir.dt.bfloat16
I16 = mybir.dt.int16
AF = mybir.ActivationFunctionType
ALU = mybir.AluOpType
P = 128
NSC = 4  # source chunks (== n sub-AllGathers == n SWDGE queues)


@dataclass(frozen=True)
class Cfg:
    N: int          # nodes
    E: int          # edges
    C: int          # channels (64)
    H: int          # mlp hidden (32)
    K: int          # poly degree (5)
    ncores: int     # 8
    piece_tok: int  # idxs per dma_gather piece
    scratch: int    # SWDGE descriptor carveout bytes
    c_need: int     # 128-token chunks per (core, src-chunk, dest-block)

    @property
    def R(self):  # rows per core
        return self.N // self.ncores

    @property
    def NB(self):  # dest blocks per core
        return math.ceil(self.R / P)

    @property
    def tail(self):  # real rows in last block
        return self.R - (self.NB - 1) * P

    @property
    def chunk_blocks(self):  # NB split into NSC block-aligned chunks
        base, rem = divmod(self.NB, NSC)
        return [base + 1] * rem + [base] * (NSC - rem)

    @property
    def chunk_rows(self):  # real rows per chunk (tail only in last)
        cb = self.chunk_blocks
        rows = [b * P for b in cb]
        rows[-1] = self.R - sum(cb[:-1]) * P
        return rows

    @property
    def chunk_start_blocks(self):
        cb = self.chunk_blocks
        return [sum(cb[:i]) for i in range(NSC)]

    @property
    def chunk_start_rows(self):
        return [sb * P for sb in self.chunk_start_blocks]

    @property
    def cap(self):  # tokens per (src-chunk, dest-block)
        return self.c_need * P

    @property
    def ncc(self):  # chunk columns per dest block
        return NSC * self.c_need

    @property
    def LQ(self):  # tokens per (core, src-chunk) stream
        return self.NB * self.cap

    @property
    def LQP(self):  # padded stream length (x piece_tok)
        return math.ceil(self.LQ / self.piece_tok) * self.piece_tok

    @property
    def n_pieces(self):
        return self.LQP // self.piece_tok

    @property
    def nchunkcol(self):  # total chunk columns in dcols
        return self.NB * self.ncc


FULL = dict(
    N=100000, E=1600000, C=64, H=32, K=5, ncores=8, piece_tok=1024, scratch=16384
)


def _preprocess(x, edge_index, cfg_kw):
    """Host-side index preprocessing -> per-core input maps + Cfg."""
    N, E, ncores = cfg_kw["N"], cfg_kw["E"], cfg_kw["ncores"]
    R = N // ncores
    NB = math.ceil(R / P)
    base, rem = divmod(NB, NSC)
    cblocks = [base + 1] * rem + [base] * (NSC - rem)
    cstartb = [sum(cblocks[:i]) for i in range(NSC)]
    cstartr = np.array([sb * P for sb in cstartb], dtype=np.int64)
    crows = [b * P for b in cblocks]
    crows[-1] = R - sum(cblocks[:-1]) * P
    crows = np.array(crows, dtype=np.int64)

    row = np.asarray(edge_index[0], dtype=np.int64)
    col = np.asarray(edge_index[1], dtype=np.int64)
    deg = np.bincount(col, minlength=N).astype(np.float32)

    m = row // R
    b = (row % R) // P
    dl = (row % R) % P
    mo = col // R
    lr = col % R
    csrc = np.digitize(lr, cstartr[1:])  # 0..NSC-1
    lidx = (mo * crows[csrc] + (lr - cstartr[csrc])).astype(np.int16)

    ngroups = ncores * NSC * NB
    key = (m * NSC + csrc) * NB + b
    order = np.lexsort((lidx, key))  # group-major, src-address-sorted
    counts = np.bincount(key, minlength=ngroups)
    c_need = int(math.ceil(counts.max() / P))
    cap = c_need * P

    starts = np.zeros(ngroups, dtype=np.int64)
    np.cumsum(counts[:-1], out=starts[1:])
    rank = np.arange(E, dtype=np.int64) - np.repeat(starts, counts)
    pos = np.repeat(np.arange(ngroups, dtype=np.int64) * cap, counts) + rank

    idx_all = np.zeros(ngroups * cap, dtype=np.int16)
    dst_all = np.full(ngroups * cap, 255.0, dtype=np.float32)
    idx_all[pos] = lidx[order]
    dst_all[pos] = dl[order].astype(np.float32)

    cfg = Cfg(c_need=c_need, **cfg_kw)
    LQ, LQP = cfg.LQ, cfg.LQP

    # dcols: [(m), P, NB*ncc] with column index gb*ncc + c*c_need + j
    dst_all = (
        dst_all.reshape(ncores, NSC, NB, c_need, P)
        .transpose(0, 2, 1, 3, 4)
        .reshape(ncores, NB * NSC * c_need, P)
    )
    idx_all = idx_all.reshape(ncores, NSC, LQ)
    if LQP > LQ:
        idx_all = np.concatenate(
            [idx_all, np.zeros((ncores, NSC, LQP - LQ), np.int16)], axis=2
        )

    in_maps = []
    for mm in range(ncores):
        # wrapped int16 indices: token i of stream c -> [i%16, i//16]; replicate
        # the 16-partition pattern across all 128 partitions (8 gpsimd cores).
        wr = np.concatenate(
            [
                np.tile(idx_all[mm, cc].reshape(LQP // 16, 16).T, (8, 1))
                for cc in range(NSC)
            ],
            axis=1,
        )  # [128, NSC*LQP//16]
        degp = np.ones(NB * P, dtype=np.float32)
        degp[:R] = deg[mm * R : (mm + 1) * R]
        in_maps.append(
            {
                "xs": np.ascontiguousarray(x[mm * R : (mm + 1) * R]).astype(np.float32),
                "degp": np.ascontiguousarray(degp.reshape(NB, P).T),
                "dcols": np.ascontiguousarray(dst_all[mm].T).astype(
                    np.float32
                ),  # converted to bf16 tile on device load? no — host bf16 below
                "idx": np.ascontiguousarray(wr),
            }
        )
    # dcols must be bf16 on device; ml_dtypes for host conversion
    try:
        from ml_dtypes import bfloat16 as _bf16

        for im in in_maps:
            im["dcols"] = im["dcols"].astype(_bf16)
    except ImportError:
        # fall back: keep f32 tensor param (device tile declared f32 then?)
        raise
    return cfg, in_maps


def _build_program(cfg: Cfg):
    nc = bacc.Bacc(
        "TRN2",
        num_swdge_queues=NSC,
        dynamic_dma_scratch_size=cfg.scratch,
    )
    C, NB, R, K = cfg.C, cfg.NB, cfg.R, cfg.K
    cneed, cap, ncc = cfg.c_need, cfg.cap, cfg.ncc
    cblocks = cfg.chunk_blocks
    cstartb = cfg.chunk_start_blocks
    crows = cfg.chunk_rows
    cstartr = cfg.chunk_start_rows

    xs_t = nc.declare_dram_parameter("xs", [R, C], F32, isOutput=False)
    degp_t = nc.declare_dram_parameter("degp", [P, NB], F32, isOutput=False)
    dcols_t = nc.declare_dram_parameter("dcols", [P, cfg.nchunkcol], BF16, isOutput=False)
    idx_t = nc.declare_dram_parameter(
        "idx", [P, NSC * (cfg.LQP // 16)], I16, isOutput=False
    )
    w1t_t = nc.declare_dram_parameter("w1t", [C + 4, cfg.H], F32, isOutput=False)
    b1c_t = nc.declare_dram_parameter("b1c", [cfg.H, 1], F32, isOutput=False)
    w2t_t = nc.declare_dram_parameter("w2t", [cfg.H, K + 1], F32, isOutput=False)
    b2r_t = nc.declare_dram_parameter("b2r", [1, K + 1], F32, isOutput=False)
    out_t = nc.declare_dram_parameter("out", [R, C], F32, isOutput=True)

    # per-step, per-chunk local slices and replicated tables (s_K lives in
    # SBUF only — the final combine is fused into step K)
    s_chk = [
        [nc.dram_tensor(f"s{k}c{c}", [crows[c], C], F32) for c in range(NSC)]
        for k in range(K)
    ]
    table = [
        [
            nc.dram_tensor(
                f"tbl{k}c{c}", [cfg.ncores * crows[c], C], F32, addr_space="Shared"
            )
            for c in range(NSC)
        ]
        for k in range(K)
    ]
    stats_loc = nc.dram_tensor("stats_loc", [136], F32)
    stats_red = nc.dram_tensor("stats_red", [136], F32, addr_space="Shared")

    groups = [list(range(cfg.ncores))]

    def chunk_views(t, c):
        """[crows[c], C] dram slice of t viewed p-major (+ tail view if any)."""
        nfull = crows[c] // P
        lo = cstartr[c]
        main = None
        if nfull > 0:
            main = t[lo : lo + nfull * P, :].rearrange("(b p) f -> p b f", p=P)
        tl = None
        if crows[c] > nfull * P:
            tl = t[lo + nfull * P : lo + crows[c], :]
        return main, tl, nfull

    def own_views(t, c):
        """views of a per-chunk dram tensor [crows[c], C]."""
        nfull = crows[c] // P
        main = None
        if nfull > 0:
            main = t[0 : nfull * P, :].rearrange("(b p) f -> p b f", p=P)
        tl = None
        if crows[c] > nfull * P:
            tl = t[nfull * P : crows[c], :]
        return main, tl, nfull

    with tile.TileContext(nc) as tc:
        with (
            tc.tile_pool(name="const", bufs=1) as cpool,
            tc.tile_pool(name="stage", bufs=8) as stpool,
            tc.tile_pool(name="graw", bufs=6) as gpool,
            tc.tile_pool(name="gbf", bufs=12) as gbpool,
            tc.tile_pool(name="sp", bufs=3) as spool,
            tc.tile_pool(name="small", bufs=2) as smpool,
            tc.tile_pool(name="pmain", bufs=4, space="PSUM") as pmain,
            tc.tile_pool(name="psmall", bufs=2, space="PSUM") as psmall,
        ):
            # ---- constants ----
            iota_t = cpool.tile([P, ncc * P], BF16)
            nc.gpsimd.iota(
                iota_t[:],
                [[0, ncc], [1, P]],
                channel_multiplier=0,
                allow_small_or_imprecise_dtypes=True,
            )
            ident64 = cpool.tile([C, C], F32)
            make_identity(nc, ident64[:])
            ones_col = cpool.tile([P, 1], F32)
            nc.gpsimd.memset(ones_col[:], 1.0)
            ones_row = cpool.tile([1, P], F32)
            nc.gpsimd.memset(ones_row[:], 1.0)

            idxs = cpool.tile([P, NSC * (cfg.LQP // 16)], I16)
            nc.sync.dma_start(out=idxs[:], in_=idx_t[:])
            dcols = cpool.tile([P, cfg.nchunkcol], BF16)
            nc.sync.dma_start(out=dcols[:], in_=dcols_t[:])
            degp = cpool.tile([P, NB], F32)
            nc.sync.dma_start(out=degp[:], in_=degp_t[:])
            w1t = cpool.tile([C + 4, cfg.H], F32)
            nc.sync.dma_start(out=w1t[:], in_=w1t_t[:])
            b1c = cpool.tile([cfg.H, 1], F32)
            nc.sync.dma_start(out=b1c[:], in_=b1c_t[:])
            w2t = cpool.tile([cfg.H, K + 1], F32)
            nc.sync.dma_start(out=w2t[:], in_=w2t_t[:])
            b2r = cpool.tile([1, K + 1], F32)
            nc.sync.dma_start(out=b2r[:], in_=b2r_t[:])

            # dis = min(deg^-0.5, 1e6); dis2 = dis^2; rdis = 1/dis
            dis = cpool.tile([P, NB], F32)
            nc.scalar.activation(dis[:], degp[:], AF.Sqrt)
            nc.vector.tensor_scalar_max(dis[:], dis[:], 1.0e-6)
            nc.vector.reciprocal(dis[:], dis[:])
            dis2 = cpool.tile([P, NB], F32)
            nc.vector.tensor_tensor(dis2[:], dis[:], dis[:], op=ALU.mult)
            rdis = cpool.tile([P, NB], F32)
            nc.vector.reciprocal(rdis[:], dis[:])

            # ---- phase A: per chunk load x, stats partials, s0, AG ----
            csum_ps = psmall.tile([P, C], F32, tag="sm")
            sq_ps = psmall.tile([P, C], F32, tag="sm2")
            for c in range(NSC):
                nb_c = cblocks[c]
                x_c = stpool.tile([P, nb_c * C], F32, tag="stage")
                xm, xtl, nfull = chunk_views(xs_t, c)
                if nfull < nb_c:
                    nc.gpsimd.memset(x_c[:, nfull * C :], 0.0)
                if nfull > 0:
                    nc.sync.dma_start(
                        out=x_c[:, 0 : nfull * C].rearrange("p (b f) -> p b f", f=C),
                        in_=xm,
                    )
                if xtl is not None:
                    nc.sync.dma_start(
                        out=x_c[0 : cfg.tail, nfull * C : (nfull + 1) * C], in_=xtl
                    )
                s_c = stpool.tile([P, nb_c * C], F32, tag="stage")
                for lb in range(nb_c):
                    gb = cstartb[c] + lb
                    nc.tensor.matmul(
                        csum_ps[0:C, 0:1],
                        lhsT=x_c[:, lb * C : (lb + 1) * C],
                        rhs=ones_col[:],
                        start=(gb == 0),
                        stop=(gb == NB - 1),
                    )
                    nc.tensor.matmul(
                        sq_ps[0:C, 0:C],
                        lhsT=x_c[:, lb * C : (lb + 1) * C],
                        rhs=x_c[:, lb * C : (lb + 1) * C],
                        start=(gb == 0),
                        stop=(gb == NB - 1),
                    )
                    # s0 = dis * x
                    nc.scalar.activation(
                        s_c[:, lb * C : (lb + 1) * C],
                        x_c[:, lb * C : (lb + 1) * C],
                        AF.Copy,
                        scale=dis[:, gb : gb + 1],
                    )
                sm, stl, nfull2 = own_views(s_chk[0][c], c)
                if nfull2 > 0:
                    nc.sync.dma_start(
                        out=sm,
                        in_=s_c[:, 0 : nfull2 * C].rearrange("p (b f) -> p b f", f=C),
                    )
                if stl is not None:
                    nc.sync.dma_start(
                        out=stl, in_=s_c[0 : cfg.tail, nfull2 * C : (nfull2 + 1) * C]
                    )
                nc.gpsimd.collective_compute(
                    "AllGather",
                    ALU.bypass,
                    replica_groups=groups,
                    ins=[s_chk[0][c][:]],
                    outs=[table[0][c][:]],
                )

            csum_sb = smpool.tile([C, 1], F32)
            nc.vector.tensor_copy(csum_sb[:], csum_ps[0:C, 0:1])
            sqd = smpool.tile([C, C], F32)
            nc.vector.tensor_tensor(sqd[:], sq_ps[0:C, 0:C], ident64[:], op=ALU.mult)
            sqch = smpool.tile([C, 1], F32)
            nc.vector.tensor_reduce(sqch[:], sqd[:], axis=mybir.AxisListType.X, op=ALU.add)

            zpad = smpool.tile([1, 8], F32, tag="zp")
            nc.gpsimd.memset(zpad[:], 0.0)
            nc.sync.dma_start(out=stats_loc[0:C], in_=csum_sb[:])
            nc.sync.dma_start(out=stats_loc[C : 2 * C], in_=sqch[:])
            nc.sync.dma_start(out=stats_loc[2 * C : 2 * C + 8], in_=zpad[:])
            nc.gpsimd.collective_compute(
                "AllReduce",
                ALU.add,
                replica_groups=groups,
                ins=[stats_loc[:]],
                outs=[stats_red[:]],
            )

            # ---- coeff MLP (overlaps diffusion steps) ----
            red = smpool.tile([1, 136], F32)
            nc.sync.dma_start(out=red[:], in_=stats_red[:])
            cin = smpool.tile([P, 1], F32, tag="cin")
            nc.sync.dma_start(out=cin[0:C, 0:1], in_=red[0:1, 0:C])
            nc.vector.tensor_scalar_mul(cin[0:C, 0:1], cin[0:C, 0:1], 1.0 / cfg.N)
            M = float(cfg.N * cfg.C)
            mean = smpool.tile([1, 1], F32, tag="m1")
            nc.vector.tensor_reduce(
                mean[:], red[0:1, 0:C], axis=mybir.AxisListType.X, op=ALU.add
            )
            nc.scalar.mul(mean[:], mean[:], 1.0 / M)
            sqred = smpool.tile([1, 1], F32, tag="m2")
            nc.vector.tensor_reduce(
                sqred[:], red[0:1, C : 2 * C], axis=mybir.AxisListType.X, op=ALU.add
            )
            msq = smpool.tile([1, 1], F32, tag="m3")
            nc.vector.tensor_tensor(msq[:], mean[:], mean[:], op=ALU.mult)
            nc.scalar.mul(msq[:], msq[:], -M)
            nc.vector.tensor_tensor(msq[:], sqred[:], msq[:], op=ALU.add)
            nc.scalar.mul(msq[:], msq[:], 1.0 / (M - 1.0))
            nc.scalar.activation(msq[:], msq[:], AF.Sqrt)  # std
            srow = smpool.tile([1, 4], F32, tag="m4")
            nc.vector.tensor_copy(srow[0:1, 0:1], mean[:])
            nc.vector.tensor_copy(srow[0:1, 1:2], msq[:])
            nc.gpsimd.memset(srow[0:1, 2:3], float(cfg.N))
            nc.gpsimd.memset(srow[0:1, 3:4], float(cfg.E))
            nc.gpsimd.dma_start(out=cin[C : C + 4, 0:1], in_=srow[:])

            h_ps = psmall.tile([P, C], F32, tag="sm")
            nc.tensor.matmul(
                h_ps[0 : cfg.H, 0:1], lhsT=w1t[:], rhs=cin[0 : C + 4, 0:1],
                start=True, stop=True,
            )
            h_sb = smpool.tile([cfg.H, 1], F32, tag="h")
            nc.scalar.activation(h_sb[:], h_ps[0 : cfg.H, 0:1], AF.Relu, bias=b1c[:])
            c_ps = psmall.tile([P, C], F32, tag="sm")
            nc.tensor.matmul(
                c_ps[0:1, 0 : K + 1], lhsT=h_sb[:], rhs=w2t[:], start=True, stop=True
            )
            z = smpool.tile([1, K + 1], F32, tag="z")
            nc.vector.tensor_tensor(z[:], c_ps[0:1, 0 : K + 1], b2r[:], op=ALU.add)
            zmax = smpool.tile([1, 1], F32, tag="m5")
            nc.vector.tensor_reduce(zmax[:], z[:], axis=mybir.AxisListType.X, op=ALU.max)
            nc.vector.tensor_scalar(
                z[:], z[:], zmax[0:1, 0:1], None, op0=ALU.subtract
            )
            nc.scalar.activation(z[:], z[:], AF.Exp)
            zsum = smpool.tile([1, 1], F32, tag="m6")
            nc.vector.tensor_reduce(zsum[:], z[:], axis=mybir.AxisListType.X, op=ALU.add)
            nc.vector.reciprocal(zsum[:], zsum[:])
            nc.vector.tensor_scalar_mul(z[:], z[:], zsum[0:1, 0:1])
            cb_ps = psmall.tile([P, C], F32, tag="sm")
            nc.tensor.matmul(
                cb_ps[:, 0 : K + 1], lhsT=ones_row[:], rhs=z[:], start=True, stop=True
            )
            c_bc = cpool.tile([P, K + 1], F32)
            nc.vector.tensor_copy(c_bc[:], cb_ps[:, 0 : K + 1])

            # ---- phase B: K diffusion steps ----
            LQ16 = cfg.LQP // 16
            pt16 = cfg.piece_tok // 16
            pt_sub = cfg.piece_tok // P  # 128-token sub-chunks per piece
            for k in range(1, K + 1):
                # gathers: i-major across chunks to match consumption order
                pieces = [[None] * cfg.n_pieces for _ in range(NSC)]
                for i in range(cfg.n_pieces):
                    for c in range(NSC):
                        gt = gpool.tile([P, pt_sub * C], F32, tag="g")
                        nc.gpsimd.dma_gather(
                            gt[:].rearrange("p (s f) -> p s f", f=C),
                            table[k - 1][c][:],
                            idxs[:, c * LQ16 + i * pt16 : c * LQ16 + (i + 1) * pt16],
                            num_idxs=cfg.piece_tok,
                            num_idxs_reg=cfg.piece_tok,
                            elem_size=C,
                            queue_num=c,
                        )
                        gb16 = gbpool.tile([P, pt_sub * C], BF16, tag="gb")
                        nc.scalar.activation(gb16[:], gt[:], AF.Copy)
                        pieces[c][i] = gb16

                for c in range(NSC):
                    nb_c = cblocks[c]
                    s_c = stpool.tile([P, nb_c * C], F32, tag="stage")
                    for lb in range(nb_c):
                        gb = cstartb[c] + lb
                        ps = pmain.tile([P, C], F32, tag="ps")
                        S = spool.tile([P, ncc * P], BF16, tag="S")
                        nc.vector.tensor_tensor(
                            S[:].rearrange("p (q f) -> p q f", f=P),
                            dcols[:, gb * ncc : (gb + 1) * ncc].to_broadcast(
                                [P, ncc, P]
                            ),
                            iota_t[:].rearrange("p (q f) -> p q f", f=P),
                            op=ALU.is_equal,
                        )
                        for sc in range(NSC):
                            for j in range(cneed):
                                t0 = gb * cap + j * P
                                gt = pieces[sc][t0 // cfg.piece_tok]
                                gv = gt[:].rearrange("p (s f) -> p s f", f=C)
                                nc.tensor.matmul(
                                    ps[:],
                                    lhsT=S[
                                        :,
                                        (sc * cneed + j) * P : (sc * cneed + j + 1) * P,
                                    ],
                                    rhs=gv[:, (t0 % cfg.piece_tok) // P, :],
                                    start=(sc == 0 and j == 0),
                                    stop=(sc == NSC - 1 and j == cneed - 1),
                                )
                        # s_k = dis^2 * a_k
                        nc.scalar.activation(
                            s_c[:, lb * C : (lb + 1) * C],
                            ps[:],
                            AF.Copy,
                            scale=dis2[:, gb : gb + 1],
                        )
                    if k < K:
                        sm, stl, nfull = own_views(s_chk[k][c], c)
                        if nfull > 0:
                            nc.sync.dma_start(
                                out=sm,
                                in_=s_c[:, 0 : nfull * C].rearrange(
                                    "p (b f) -> p b f", f=C
                                ),
                            )
                        if stl is not None:
                            nc.sync.dma_start(
                                out=stl,
                                in_=s_c[0 : cfg.tail, nfull * C : (nfull + 1) * C],
                            )
                        nc.gpsimd.collective_compute(
                            "AllGather",
                            ALU.bypass,
                            replica_groups=groups,
                            ins=[s_chk[k][c][:]],
                            outs=[table[k][c][:]],
                        )
                        continue

                    # ---- k == K: fuse the final combine for this chunk ----
                    acc = stpool.tile([P, nb_c * C], F32, tag="stage")
                    nc.vector.tensor_scalar_mul(acc[:], s_c[:], c_bc[:, K : K + 1])
                    for kk in range(1, K):
                        sl = stpool.tile([P, nb_c * C], F32, tag="stage")
                        sm, stl, nfull = own_views(s_chk[kk][c], c)
                        if nfull < nb_c:
                            nc.gpsimd.memset(sl[:, nfull * C :], 0.0)
                        if nfull > 0:
                            nc.sync.dma_start(
                                out=sl[:, 0 : nfull * C].rearrange(
                                    "p (b f) -> p b f", f=C
                                ),
                                in_=sm,
                            )
                        if stl is not None:
                            nc.sync.dma_start(
                                out=sl[0 : cfg.tail, nfull * C : (nfull + 1) * C],
                                in_=stl,
                            )
                        nc.vector.tensor_scalar_mul(sl[:], sl[:], c_bc[:, kk : kk + 1])
                        nc.vector.tensor_tensor(acc[:], acc[:], sl[:], op=ALU.add)
                    for lb in range(nb_c):
                        gb = cstartb[c] + lb
                        nc.vector.tensor_scalar_mul(
                            acc[:, lb * C : (lb + 1) * C],
                            acc[:, lb * C : (lb + 1) * C],
                            rdis[:, gb : gb + 1],
                        )
                    xl = stpool.tile([P, nb_c * C], F32, tag="stage")
                    xm, xtl, nfull = chunk_views(xs_t, c)
                    if nfull < nb_c:
                        nc.gpsimd.memset(xl[:, nfull * C :], 0.0)
                    if nfull > 0:
                        nc.sync.dma_start(
                            out=xl[:, 0 : nfull * C].rearrange("p (b f) -> p b f", f=C),
                            in_=xm,
                        )
                    if xtl is not None:
                        nc.sync.dma_start(
                            out=xl[0 : cfg.tail, nfull * C : (nfull + 1) * C], in_=xtl
                        )
                    nc.vector.tensor_scalar_mul(xl[:], xl[:], c_bc[:, 0:1])
                    nc.vector.tensor_tensor(acc[:], acc[:], xl[:], op=ALU.add)

                    om, otl, nfull = chunk_views(out_t, c)
                    if nfull > 0:
                        nc.sync.dma_start(
                            out=om,
                            in_=acc[:, 0 : nfull * C].rearrange("p (b f) -> p b f", f=C),
                        )
                    if otl is not None:
                        nc.sync.dma_start(
                            out=otl, in_=acc[0 : cfg.tail, nfull * C : (nfull + 1) * C]
                        )

    nc.finalize()
    return nc


_CACHE = {}


def _get_program(cfg: Cfg):
    if cfg not in _CACHE:
        _CACHE[cfg] = _build_program(cfg)
    return _CACHE[cfg]


def _run(inputs, trace=False, cfg_kw=None):
    from concourse.bass_utils import run_bass_kernel_spmd

    cfg_kw = dict(cfg_kw or FULL)
    x = np.asarray(inputs["x"], dtype=np.float32)
    cfg, in_maps = _preprocess(x, inputs["edge_index"], cfg_kw)
    W1 = np.asarray(inputs["W1"], dtype=np.float32)
    b1 = np.asarray(inputs["b1"], dtype=np.float32)
    W2 = np.asarray(inputs["W2"], dtype=np.float32)
    b2 = np.asarray(inputs["b2"], dtype=np.float32)
    for im in in_maps:
        im["w1t"] = np.ascontiguousarray(W1.T)
        im["b1c"] = np.ascontiguousarray(b1[:, None])
        im["w2t"] = np.ascontiguousarray(W2.T)
        im["b2r"] = np.ascontiguousarray(b2[None, :])
    nc = _get_program(cfg)
    res = run_bass_kernel_spmd(
        nc, in_maps, core_ids=list(range(cfg.ncores)), trace=trace
    )
    out = np.concatenate([res.results[i]["out"] for i in range(cfg.ncores)], axis=0)
    return out, res.exec_time_ns


def kernel(**inputs) -> np.ndarray:
    out, _ = _run(inputs)
    return out


# ---------------------------------------------------------------------------
# toy-scale validation against a numpy port of the reference, via CoreSim
# ---------------------------------------------------------------------------


def _np_reference(x, edge_index, W1, b1, W2, b2, K=5):
    N, C = x.shape
    E = edge_index.shape[1]
    row, col = edge_index[0].astype(np.int64), edge_index[1].astype(np.int64)
    deg = np.bincount(col, minlength=N).astype(np.float32)
    with np.errstate(divide="ignore"):
        dis = np.minimum(deg ** -0.5, 1e6).astype(np.float32)
    norm = dis[row] * dis[col]
    xm = x.mean(axis=0)
    stats = np.array([x.mean(), x.std(ddof=1), N, E], dtype=np.float32)
    cin = np.concatenate([xm, stats])
    h = np.maximum(W1 @ cin + b1, 0.0)
    zz = W2 @ h + b2
    zz = np.exp(zz - zz.max())
    coeffs = zz / zz.sum()
    result = coeffs[0] * x
    tx = x.copy()
    for k in range(1, K + 1):
        nt = np.zeros_like(tx)
        np.add.at(nt, row, norm[:, None] * tx[col])
        tx = nt
        result = result + coeffs[k] * tx
    return result


def _selftest_sim():
    from concourse.bass_interp import MultiCoreSim

    rng = np.random.default_rng(0)
    kw = dict(
        N=6400, E=25600, C=64, H=32, K=5, ncores=8, piece_tok=256, scratch=16384
    )
    x = rng.standard_normal((kw["N"], kw["C"])).astype(np.float32)
    ei = rng.integers(0, kw["N"], size=(2, kw["E"])).astype(np.int32)
    W1 = rng.uniform(-1, 1, (kw["H"], kw["C"] + 4)).astype(np.float32) / 8
    b1 = rng.uniform(-1, 1, (kw["H"],)).astype(np.float32) / 8
    W2 = rng.uniform(-1, 1, (kw["K"] + 1, kw["H"])).astype(np.float32) / 5
    b2 = rng.uniform(-1, 1, (kw["K"] + 1,)).astype(np.float32) / 5

    cfg, in_maps = _preprocess(x, ei, kw)
    print("toy cfg:", cfg)
    for im in in_maps:
        im["w1t"] = np.ascontiguousarray(W1.T)
        im["b1c"] = np.ascontiguousarray(b1[:, None])
        im["w2t"] = np.ascontiguousarray(W2.T)
        im["b2r"] = np.ascontiguousarray(b2[None, :])
    nc = _build_program(cfg)
    sim = MultiCoreSim(nc, cfg.ncores)
    for i in range(cfg.ncores):
        for name, arr in in_maps[i].items():
            sim.cores[i].tensor(name)[:] = arr
    sim.simulate()
    out = np.concatenate(
        [sim.cores[i].tensor("out") for i in range(cfg.ncores)], axis=0
    )
    exp = _np_reference(x, ei, W1, b1, W2, b2, K=kw["K"])
    err = np.abs(out - exp).max() / (np.abs(exp).max() + 1e-30)
    rel = np.linalg.norm(out - exp) / (np.linalg.norm(exp) + 1e-30)
    print(f"sim selftest: max-abs-rel {err:.3e}  fro-rel {rel:.3e}")
    assert rel < 5e-3, (rel, err)
    print("SIM SELFTEST PASSED")


if __name__ == "__main__":
    _selftest_sim()
